# revision 16
# baseline (speedup 1.0000x reference)
"""Expert-parallel MoE layer for 8 Trainium2 NeuronCores.

Strategy: each of the 8 experts is assigned to one core. The host computes
the routing (which tokens go to which expert and with what combined weight),
gathers + transposes each expert's tokens into a padded [D, C] activation
matrix, and each core runs a fused  gelu(x @ W1 + b1) @ W2 + b2  kernel for
its expert, scaling each token's output column by the combine weight. The
host scatter-adds the per-expert outputs back into the full [B, S, D] output.

Matmuls run as float32r (full fp32 storage, reduced-precision PE multiply) at
1 column/cycle -- 4x faster than true fp32 matmul on TRN2.
"""

import sys

if "/opt/trn_rl_repo" not in sys.path:
    sys.path.insert(0, "/opt/trn_rl_repo")

import numpy as np

import concourse.bass as bass
import concourse.tile as tile
from concourse import bacc, mybir
from concourse.bass_utils import run_bass_kernel_spmd

B, S, D, F, E, TOPK = 4, 2048, 512, 1024, 8, 2
T = B * S
F32 = mybir.dt.float32
F32R = mybir.dt.float32r

DC = D // 128  # 4 contraction chunks for x @ W1
FC = F // 128  # 8 contraction chunks for h @ W2

# Set by test harness to capture a profile; harness-invisible otherwise.
TRACE = False
LAST_RESULTS = None

_nc_cache = {}


def _token_tiles(C):
    """Split C token columns into token-tile sizes.

    The first tile is up to 1536 columns (three 512 PSUM sub-blocks): its fc
    sweep gives the PE a long stretch of work that needs only w1 + the first
    x^T slices, which is exactly the runway the DMA engines need to land w2
    and the later x^T slices before they are consumed. Later tiles are
    ~512s; every tile stays >= 256 so f32r matmul runs at full rate.
    """
    assert C % 64 == 0 and C >= 512
    first = min(1536, C)
    rem = C - first
    if rem == 0:
        return [first]
    n = -(-rem // 512)
    sizes = [first]
    base, extra = divmod(rem, n)
    for j in range(n):
        sizes.append(base + (1 if j < extra else 0))
    assert sum(sizes) == C and all(s >= 256 or s == C for s in sizes[1:]), sizes
    return sizes


def _build_nc(C):
    nc = bacc.Bacc("TRN2", num_devices=E)

    xt = nc.dram_tensor("xt", [D, C], F32, kind="ExternalInput")
    w1 = nc.dram_tensor("w1", [D, F], F32, kind="ExternalInput")
    b1 = nc.dram_tensor("b1", [F, 1], F32, kind="ExternalInput")
    w2 = nc.dram_tensor("w2", [F, D], F32, kind="ExternalInput")
    b2 = nc.dram_tensor("b2", [D, 1], F32, kind="ExternalInput")
    cw = nc.dram_tensor("cw", [128, C], F32, kind="ExternalInput")
    yt = nc.dram_tensor("yt", [D, C], F32, kind="ExternalOutput")

    # DRAM views with the 128-partition chunk dim split out
    xt_r = xt.rearrange("(c p) t -> p c t", p=128)  # [128, DC, C]
    w1_r = w1.rearrange("(c p) (g f) -> p g c f", p=128, f=128)  # [128, FC, DC, 128]
    w2_r = w2.rearrange("(c p) (g d) -> p g c d", p=128, d=128)  # [128, DC, FC, 128]
    b1_r = b1.rearrange("(c p) o -> p (c o)", p=128)  # [128, FC]
    b2_r = b2.rearrange("(c p) o -> p (c o)", p=128)  # [128, DC]
    yt_r = yt.rearrange("(c p) t -> p c t", p=128)  # [128, DC, C]

    sizes = _token_tiles(C)

    with tile.TileContext(nc) as tc:
        with (
            tc.tile_pool(name="consts", bufs=1) as consts,
            tc.tile_pool(name="xtp", bufs=1) as xtp,
            tc.tile_pool(name="hp", bufs=26) as hp,
            tc.tile_pool(name="yp", bufs=4) as yp,
            tc.tile_pool(name="ybig", bufs=8) as ybigp,
            tc.tile_pool(name="ps_h", bufs=4, space="PSUM") as ps_h,
            tc.tile_pool(name="ps_y", bufs=4, space="PSUM") as ps_y,
        ):
            # w1 grouped by output (F) chunk so the very first matmul group
            # only needs a 256 KB slice; slices stream in ahead of the PE.
            w1_sb = consts.tile([128, FC, DC, 128], F32R, tag="w1")
            nc.sync.dma_start(
                out=w1_sb[:, 0, :, :], in_=w1_r[:, 0, :, :].bitcast(F32R)
            )

            # Sub-block split of every token tile (<=512 cols per PSUM bank);
            # x^T is loaded one sub-block at a time so the first matmul only
            # gates on the first 512 columns.
            offs = []
            off = 0
            tile_subs = []
            for i, nt in enumerate(sizes):
                offs.append(off)
                subs = []
                so = 0
                while so < nt:
                    subs.append((so, min(512, nt - so)))
                    so += 512
                tile_subs.append(subs)
                off += nt

            xt_sub = {}

            def _load_xt(i, si):
                soff, ssz = tile_subs[i][si]
                t = xtp.tile(
                    [128, DC, ssz], F32R, tag=f"xt{i}_{si}", name=f"xt_sb{i}_{si}"
                )
                base = offs[i] + soff
                nc.sync.dma_start(
                    out=t[:, :, :],
                    in_=xt_r[:, :, base:base + ssz].bitcast(F32R),
                )
                xt_sub[(i, si)] = t

            _load_xt(0, 0)

            for g in range(1, FC):
                nc.sync.dma_start(
                    out=w1_sb[:, g, :, :], in_=w1_r[:, g, :, :].bitcast(F32R)
                )

            for si in range(1, len(tile_subs[0])):
                _load_xt(0, si)

            # w2 grouped by output (D) chunk; needed right after the first
            # fc sweep, so it goes ahead of the remaining x^T tiles in the
            # sync HWDGE FIFO.
            w2_sb = consts.tile([128, DC, FC, 128], F32R, tag="w2")
            for g in range(DC):
                nc.sync.dma_start(
                    out=w2_sb[:, g, :, :], in_=w2_r[:, g, :, :].bitcast(F32R)
                )

            # biases ride the ACT HWDGE ring, off the critical sync FIFO
            b1_sb = consts.tile([128, FC], F32, tag="b1")
            nc.scalar.dma_start(out=b1_sb[:, :], in_=b1_r[:, :])
            b2_sb = consts.tile([128, DC], F32, tag="b2")
            nc.scalar.dma_start(out=b2_sb[:, :], in_=b2_r[:, :])

            # combine weights (host-replicated to 128 partitions); needed
            # first by tile0's dc sweep, so it follows w2 in the sync FIFO
            cwb = consts.tile([128, C], F32, tag="cw")
            nc.sync.dma_start(out=cwb[:, :], in_=cw[:, :])

            for i in range(1, len(sizes)):
                for si in range(len(tile_subs[i])):
                    _load_xt(i, si)

            for i, nt in enumerate(sizes):
                off = offs[i]
                subs = tile_subs[i]

                # h^T tile per (sub-block, F-chunk): [128 f, ssz tokens]
                h_tiles = {}
                for si, (soff, ssz) in enumerate(subs):
                    for fc in range(FC):
                        ps = ps_h.tile([128, ssz], F32, tag="psh")
                        for dc in range(DC):
                            nc.tensor.matmul(
                                ps[:, :],
                                w1_sb[:, fc, dc, :],
                                xt_sub[(i, si)][:, dc, :],
                                start=(dc == 0),
                                stop=(dc == DC - 1),
                            )
                        h = hp.tile([128, ssz], F32R, tag="h")
                        nc.scalar.activation(
                            h[:, :], ps[:, :],
                            mybir.ActivationFunctionType.Gelu_apprx_tanh,
                            bias=b1_sb[:, fc:fc + 1],
                        )
                        h_tiles[(si, fc)] = h

                for si, (soff, ssz) in enumerate(subs):
                    for dc in range(DC):
                        ps2 = ps_y.tile([128, ssz], F32, tag="psy")
                        for fc in range(FC):
                            nc.tensor.matmul(
                                ps2[:, :],
                                w2_sb[:, dc, fc, :],
                                h_tiles[(si, fc)][:, :],
                                start=(fc == 0),
                                stop=(fc == FC - 1),
                            )
                        ytmp = yp.tile([128, ssz], F32, tag="ytmp")
                        nc.scalar.activation(
                            ytmp[:, :], ps2[:, :],
                            mybir.ActivationFunctionType.Identity,
                            bias=b2_sb[:, dc:dc + 1],
                        )
                        yout = ybigp.tile([128, ssz], F32, tag="yout")
                        nc.vector.tensor_mul(
                            yout[:, :], ytmp[:, :],
                            cwb[:, off + soff:off + soff + ssz],
                        )
                        nc.sync.dma_start(
                            out=yt_r[:, dc, off + soff:off + soff + ssz],
                            in_=yout[:, :],
                        )

    nc.finalize()
    return nc


def kernel(hidden, top_k_indices, top_k_weights, W1, b1, W2, b2):
    global LAST_RESULTS
    x = np.ascontiguousarray(np.asarray(hidden, dtype=np.float32).reshape(T, D))
    idx = np.asarray(top_k_indices).reshape(T, TOPK)
    w = np.asarray(top_k_weights, dtype=np.float32).reshape(T, TOPK)
    W1 = np.asarray(W1, dtype=np.float32)
    b1 = np.asarray(b1, dtype=np.float32)
    W2 = np.asarray(W2, dtype=np.float32)
    b2 = np.asarray(b2, dtype=np.float32)

    # Host routing: token lists + combined weights per expert
    tok_lists, cw_lists = [], []
    for e in range(E):
        m = idx == e
        toks = np.nonzero(m.any(axis=1))[0]
        cw_t = (w * m).sum(axis=1)[toks]
        tok_lists.append(toks)
        cw_lists.append(cw_t)

    maxn = max(len(t) for t in tok_lists)
    C = max(512, -(-maxn // 64) * 64)

    if C not in _nc_cache:
        _nc_cache[C] = _build_nc(C)
    nc = _nc_cache[C]

    in_maps = []
    for e in range(E):
        toks = tok_lists[e]
        n = len(toks)
        xt = np.zeros((D, C), np.float32)
        xt[:, :n] = x[toks].T
        cw_arr = np.zeros((128, C), np.float32)
        cw_arr[:, :n] = cw_lists[e][None, :]
        in_maps.append({
            "xt": xt,
            "w1": np.ascontiguousarray(W1[e]),
            "b1": np.ascontiguousarray(b1[e].reshape(F, 1)),
            "w2": np.ascontiguousarray(W2[e]),
            "b2": np.ascontiguousarray(b2[e].reshape(D, 1)),
            "cw": cw_arr,
        })

    kwargs = {}
    if TRACE:
        kwargs = dict(trace=True, trace_cores=list(range(E)))
    res = run_bass_kernel_spmd(nc, in_maps, core_ids=list(range(E)), **kwargs)
    LAST_RESULTS = res

    out = np.zeros((T, D), np.float32)
    for e in range(E):
        toks = tok_lists[e]
        n = len(toks)
        out[toks] += res.results[e]["yt"][:, :n].T
    return out.reshape(B, S, D)


# revision 17
# speedup vs baseline: 1.0768x; 1.0768x over previous
"""Expert-parallel MoE layer for 8 Trainium2 NeuronCores.

Strategy: each of the 8 experts is assigned to one core. The host computes
the routing (which tokens go to which expert and with what combined weight),
gathers + transposes each expert's tokens into a padded [D, C] activation
matrix, and each core runs a fused  gelu(x @ W1 + b1) @ W2 + b2  kernel for
its expert, scaling each token's output column by the combine weight. The
host scatter-adds the per-expert outputs back into the full [B, S, D] output.

Matmuls run as float32r (full fp32 storage, reduced-precision PE multiply) at
1 column/cycle -- 4x faster than true fp32 matmul on TRN2.
"""

import sys

if "/opt/trn_rl_repo" not in sys.path:
    sys.path.insert(0, "/opt/trn_rl_repo")

import numpy as np

import concourse.bass as bass
import concourse.tile as tile
from concourse import bacc, mybir
from concourse.bass_utils import run_bass_kernel_spmd

B, S, D, F, E, TOPK = 4, 2048, 512, 1024, 8, 2
T = B * S
F32 = mybir.dt.float32
F32R = mybir.dt.float32r

DC = D // 128  # 4 contraction chunks for x @ W1
FC = F // 128  # 8 contraction chunks for h @ W2

# Set by test harness to capture a profile; harness-invisible otherwise.
TRACE = False
LAST_RESULTS = None

_nc_cache = {}


def _token_tiles(C):
    """Split C token columns into token-tile sizes.

    The first tile is up to 1536 columns (three 512 PSUM sub-blocks): its fc
    sweep gives the PE a long stretch of work that needs only w1 + the first
    x^T slices, which is exactly the runway the DMA engines need to land w2
    and the later x^T slices before they are consumed. Later tiles are
    ~512s; every tile stays >= 256 so f32r matmul runs at full rate.
    """
    assert C % 64 == 0 and C >= 512
    first = min(1536, C)
    rem = C - first
    if rem == 0:
        return [first]
    n = -(-rem // 512)
    sizes = [first]
    base, extra = divmod(rem, n)
    for j in range(n):
        sizes.append(base + (1 if j < extra else 0))
    assert sum(sizes) == C and all(s >= 256 or s == C for s in sizes[1:]), sizes
    return sizes


def _build_nc(C):
    nc = bacc.Bacc("TRN2", num_devices=E)

    xt = nc.dram_tensor("xt", [D, C], F32, kind="ExternalInput")
    w1 = nc.dram_tensor("w1", [D, F], F32, kind="ExternalInput")
    b1 = nc.dram_tensor("b1", [F, 1], F32, kind="ExternalInput")
    w2 = nc.dram_tensor("w2", [F, D], F32, kind="ExternalInput")
    b2 = nc.dram_tensor("b2", [D, 1], F32, kind="ExternalInput")
    cw = nc.dram_tensor("cw", [128, C], F32, kind="ExternalInput")
    yt = nc.dram_tensor("yt", [D, C], F32, kind="ExternalOutput")

    # DRAM views with the 128-partition chunk dim split out
    xt_r = xt.rearrange("(c p) t -> p c t", p=128)  # [128, DC, C]
    w1_r = w1.rearrange("(c p) f -> p c f", p=128)  # [128, DC, F]
    w2_r = w2.rearrange("(c p) d -> p c d", p=128)  # [128, FC, D]
    b1_r = b1.rearrange("(c p) o -> p (c o)", p=128)  # [128, FC]
    b2_r = b2.rearrange("(c p) o -> p (c o)", p=128)  # [128, DC]
    yt_r = yt.rearrange("(c p) t -> p c t", p=128)  # [128, DC, C]

    sizes = _token_tiles(C)

    with tile.TileContext(nc) as tc:
        with (
            tc.tile_pool(name="consts", bufs=1) as consts,
            tc.tile_pool(name="xtp", bufs=1) as xtp,
            tc.tile_pool(name="hp", bufs=26) as hp,
            tc.tile_pool(name="yp", bufs=4) as yp,
            tc.tile_pool(name="ybig", bufs=8) as ybigp,
            tc.tile_pool(name="ps_h", bufs=4, space="PSUM") as ps_h,
            tc.tile_pool(name="ps_y", bufs=4, space="PSUM") as ps_y,
        ):
            # w1: small first slice (the f-columns the first fc group needs)
            # so the first matmul gates on only 256 KB, then the rest in one
            # contiguous-run DMA at full line rate.
            w1_sb = consts.tile([128, DC, F], F32R, tag="w1")
            nc.sync.dma_start(
                out=w1_sb[:, :, 0:128], in_=w1_r[:, :, 0:128].bitcast(F32R)
            )

            # Sub-block split of every token tile (<=512 cols per PSUM bank);
            # x^T is loaded one sub-block at a time so the first matmul only
            # gates on the first 512 columns.
            offs = []
            off = 0
            tile_subs = []
            for i, nt in enumerate(sizes):
                offs.append(off)
                subs = []
                so = 0
                while so < nt:
                    subs.append((so, min(512, nt - so)))
                    so += 512
                tile_subs.append(subs)
                off += nt

            xt_sub = {}

            def _load_xt(i, si):
                soff, ssz = tile_subs[i][si]
                t = xtp.tile(
                    [128, DC, ssz], F32R, tag=f"xt{i}_{si}", name=f"xt_sb{i}_{si}"
                )
                base = offs[i] + soff
                nc.sync.dma_start(
                    out=t[:, :, :],
                    in_=xt_r[:, :, base:base + ssz].bitcast(F32R),
                )
                xt_sub[(i, si)] = t

            _load_xt(0, 0)

            nc.sync.dma_start(
                out=w1_sb[:, :, 128:F], in_=w1_r[:, :, 128:F].bitcast(F32R)
            )

            for si in range(1, len(tile_subs[0])):
                _load_xt(0, si)

            # w2; needed after the first fc sweep, so it goes ahead of the
            # remaining x^T tiles in the sync HWDGE FIFO. First d-chunk
            # separately (it unblocks the first dc group), then the rest.
            w2_sb = consts.tile([128, FC, D], F32R, tag="w2")
            nc.sync.dma_start(
                out=w2_sb[:, :, 0:128], in_=w2_r[:, :, 0:128].bitcast(F32R)
            )
            nc.sync.dma_start(
                out=w2_sb[:, :, 128:D], in_=w2_r[:, :, 128:D].bitcast(F32R)
            )

            # biases ride the ACT HWDGE ring, off the critical sync FIFO
            b1_sb = consts.tile([128, FC], F32, tag="b1")
            nc.scalar.dma_start(out=b1_sb[:, :], in_=b1_r[:, :])
            b2_sb = consts.tile([128, DC], F32, tag="b2")
            nc.scalar.dma_start(out=b2_sb[:, :], in_=b2_r[:, :])

            # combine weights (host-replicated to 128 partitions); needed
            # first by tile0's dc sweep, so it follows w2 in the sync FIFO
            cwb = consts.tile([128, C], F32, tag="cw")
            nc.sync.dma_start(out=cwb[:, :], in_=cw[:, :])

            for i in range(1, len(sizes)):
                for si in range(len(tile_subs[i])):
                    _load_xt(i, si)

            for i, nt in enumerate(sizes):
                off = offs[i]
                subs = tile_subs[i]

                # h^T tile per (sub-block, F-chunk): [128 f, ssz tokens]
                h_tiles = {}
                for si, (soff, ssz) in enumerate(subs):
                    for fc in range(FC):
                        ps = ps_h.tile([128, ssz], F32, tag="psh")
                        for dc in range(DC):
                            nc.tensor.matmul(
                                ps[:, :],
                                w1_sb[:, dc, fc * 128:(fc + 1) * 128],
                                xt_sub[(i, si)][:, dc, :],
                                start=(dc == 0),
                                stop=(dc == DC - 1),
                            )
                        h = hp.tile([128, ssz], F32R, tag="h")
                        nc.scalar.activation(
                            h[:, :], ps[:, :],
                            mybir.ActivationFunctionType.Gelu_apprx_tanh,
                            bias=b1_sb[:, fc:fc + 1],
                        )
                        h_tiles[(si, fc)] = h

                for si, (soff, ssz) in enumerate(subs):
                    for dc in range(DC):
                        ps2 = ps_y.tile([128, ssz], F32, tag="psy")
                        for fc in range(FC):
                            nc.tensor.matmul(
                                ps2[:, :],
                                w2_sb[:, fc, dc * 128:(dc + 1) * 128],
                                h_tiles[(si, fc)][:, :],
                                start=(fc == 0),
                                stop=(fc == FC - 1),
                            )
                        ytmp = yp.tile([128, ssz], F32, tag="ytmp")
                        nc.scalar.activation(
                            ytmp[:, :], ps2[:, :],
                            mybir.ActivationFunctionType.Identity,
                            bias=b2_sb[:, dc:dc + 1],
                        )
                        yout = ybigp.tile([128, ssz], F32, tag="yout")
                        nc.vector.tensor_mul(
                            yout[:, :], ytmp[:, :],
                            cwb[:, off + soff:off + soff + ssz],
                        )
                        nc.sync.dma_start(
                            out=yt_r[:, dc, off + soff:off + soff + ssz],
                            in_=yout[:, :],
                        )

    nc.finalize()
    return nc


def kernel(hidden, top_k_indices, top_k_weights, W1, b1, W2, b2):
    global LAST_RESULTS
    x = np.ascontiguousarray(np.asarray(hidden, dtype=np.float32).reshape(T, D))
    idx = np.asarray(top_k_indices).reshape(T, TOPK)
    w = np.asarray(top_k_weights, dtype=np.float32).reshape(T, TOPK)
    W1 = np.asarray(W1, dtype=np.float32)
    b1 = np.asarray(b1, dtype=np.float32)
    W2 = np.asarray(W2, dtype=np.float32)
    b2 = np.asarray(b2, dtype=np.float32)

    # Host routing: token lists + combined weights per expert
    tok_lists, cw_lists = [], []
    for e in range(E):
        m = idx == e
        toks = np.nonzero(m.any(axis=1))[0]
        cw_t = (w * m).sum(axis=1)[toks]
        tok_lists.append(toks)
        cw_lists.append(cw_t)

    maxn = max(len(t) for t in tok_lists)
    C = max(512, -(-maxn // 64) * 64)

    if C not in _nc_cache:
        _nc_cache[C] = _build_nc(C)
    nc = _nc_cache[C]

    in_maps = []
    for e in range(E):
        toks = tok_lists[e]
        n = len(toks)
        xt = np.zeros((D, C), np.float32)
        xt[:, :n] = x[toks].T
        cw_arr = np.zeros((128, C), np.float32)
        cw_arr[:, :n] = cw_lists[e][None, :]
        in_maps.append({
            "xt": xt,
            "w1": np.ascontiguousarray(W1[e]),
            "b1": np.ascontiguousarray(b1[e].reshape(F, 1)),
            "w2": np.ascontiguousarray(W2[e]),
            "b2": np.ascontiguousarray(b2[e].reshape(D, 1)),
            "cw": cw_arr,
        })

    kwargs = {}
    if TRACE:
        kwargs = dict(trace=True, trace_cores=list(range(E)))
    res = run_bass_kernel_spmd(nc, in_maps, core_ids=list(range(E)), **kwargs)
    LAST_RESULTS = res

    out = np.zeros((T, D), np.float32)
    for e in range(E):
        toks = tok_lists[e]
        n = len(toks)
        out[toks] += res.results[e]["yt"][:, :n].T
    return out.reshape(B, S, D)


# revision 18
# speedup vs baseline: 1.1033x; 1.0246x over previous
"""Expert-parallel MoE layer for 8 Trainium2 NeuronCores.

Strategy: each of the 8 experts is assigned to one core. The host computes
the routing (which tokens go to which expert and with what combined weight),
gathers + transposes each expert's tokens into a padded [D, C] activation
matrix, and each core runs a fused  gelu(x @ W1 + b1) @ W2 + b2  kernel for
its expert, scaling each token's output column by the combine weight. The
host scatter-adds the per-expert outputs back into the full [B, S, D] output.

Matmuls run as float32r (full fp32 storage, reduced-precision PE multiply) at
1 column/cycle -- 4x faster than true fp32 matmul on TRN2.
"""

import sys

if "/opt/trn_rl_repo" not in sys.path:
    sys.path.insert(0, "/opt/trn_rl_repo")

import numpy as np

import concourse.bass as bass
import concourse.tile as tile
from concourse import bacc, mybir
from concourse.bass_utils import run_bass_kernel_spmd

B, S, D, F, E, TOPK = 4, 2048, 512, 1024, 8, 2
T = B * S
F32 = mybir.dt.float32
F32R = mybir.dt.float32r

DC = D // 128  # 4 contraction chunks for x @ W1
FC = F // 128  # 8 contraction chunks for h @ W2

# Set by test harness to capture a profile; harness-invisible otherwise.
TRACE = False
LAST_RESULTS = None

_nc_cache = {}


def _token_tiles(C):
    """Split C token columns into token-tile sizes.

    The first tile is up to 1536 columns (three 512 PSUM sub-blocks): its fc
    sweep gives the PE a long stretch of work that needs only w1 + the first
    x^T slices, which is exactly the runway the DMA engines need to land w2
    and the later x^T slices before they are consumed. Later tiles are
    ~512s; every tile stays >= 256 so f32r matmul runs at full rate.
    """
    assert C % 64 == 0 and C >= 512
    first = min(1536, C)
    rem = C - first
    if rem == 0:
        return [first]
    n = -(-rem // 512)
    sizes = [first]
    base, extra = divmod(rem, n)
    for j in range(n):
        sizes.append(base + (1 if j < extra else 0))
    assert sum(sizes) == C and all(s >= 256 or s == C for s in sizes[1:]), sizes
    return sizes


def _build_nc(C):
    nc = bacc.Bacc("TRN2", num_devices=E)

    xt = nc.dram_tensor("xt", [D, C], F32, kind="ExternalInput")
    w1 = nc.dram_tensor("w1", [D, F], F32, kind="ExternalInput")
    b1 = nc.dram_tensor("b1", [F, 1], F32, kind="ExternalInput")
    w2 = nc.dram_tensor("w2", [F, D], F32, kind="ExternalInput")
    b2 = nc.dram_tensor("b2", [D, 1], F32, kind="ExternalInput")
    cw = nc.dram_tensor("cw", [128, C], F32, kind="ExternalInput")
    yt = nc.dram_tensor("yt", [D, C], F32, kind="ExternalOutput")

    # DRAM views with the 128-partition chunk dim split out
    xt_r = xt.rearrange("(c p) t -> p c t", p=128)  # [128, DC, C]
    w1_r = w1.rearrange("(c p) f -> p c f", p=128)  # [128, DC, F]
    w2_r = w2.rearrange("(c p) d -> p c d", p=128)  # [128, FC, D]
    b1_r = b1.rearrange("(c p) o -> p (c o)", p=128)  # [128, FC]
    b2_r = b2.rearrange("(c p) o -> p (c o)", p=128)  # [128, DC]
    yt_r = yt.rearrange("(c p) t -> p c t", p=128)  # [128, DC, C]

    sizes = _token_tiles(C)

    with tile.TileContext(nc) as tc:
        with (
            tc.tile_pool(name="consts", bufs=1) as consts,
            tc.tile_pool(name="xtp", bufs=1) as xtp,
            tc.tile_pool(name="hp", bufs=26) as hp,
            tc.tile_pool(name="yp", bufs=4) as yp,
            tc.tile_pool(name="ybig", bufs=8) as ybigp,
            tc.tile_pool(name="ps_h", bufs=4, space="PSUM") as ps_h,
            tc.tile_pool(name="ps_y", bufs=4, space="PSUM") as ps_y,
        ):
            # w1: small first slice (the f-columns the first fc group needs)
            # so the first matmul gates on only 256 KB, then the rest in one
            # contiguous-run DMA at full line rate.
            w1_sb = consts.tile([128, DC, F], F32R, tag="w1")
            nc.sync.dma_start(
                out=w1_sb[:, :, 0:128], in_=w1_r[:, :, 0:128].bitcast(F32R)
            )

            # Sub-block split of every token tile (<=512 cols per PSUM bank);
            # x^T is loaded one sub-block at a time so the first matmul only
            # gates on the first 512 columns.
            offs = []
            off = 0
            tile_subs = []
            for i, nt in enumerate(sizes):
                offs.append(off)
                subs = []
                so = 0
                while so < nt:
                    subs.append((so, min(512, nt - so)))
                    so += 512
                tile_subs.append(subs)
                off += nt

            xt_sub = {}

            def _load_xt(i, si):
                soff, ssz = tile_subs[i][si]
                t = xtp.tile(
                    [128, DC, ssz], F32R, tag=f"xt{i}_{si}", name=f"xt_sb{i}_{si}"
                )
                base = offs[i] + soff
                nc.sync.dma_start(
                    out=t[:, :, :],
                    in_=xt_r[:, :, base:base + ssz].bitcast(F32R),
                )
                xt_sub[(i, si)] = t

            _load_xt(0, 0)

            # staged so each slice lands just ahead of the fc group that
            # consumes it (one monolithic DMA would stall fc1 until the
            # whole rest of w1 arrived)
            for lo, hi in ((128, 256), (256, 512), (512, 1024)):
                nc.sync.dma_start(
                    out=w1_sb[:, :, lo:hi], in_=w1_r[:, :, lo:hi].bitcast(F32R)
                )

            for si in range(1, len(tile_subs[0])):
                _load_xt(0, si)

            # w2; needed after the first fc sweep, so it goes ahead of the
            # remaining x^T tiles in the sync HWDGE FIFO. First d-chunk
            # separately (it unblocks the first dc group), then the rest.
            w2_sb = consts.tile([128, FC, D], F32R, tag="w2")
            nc.sync.dma_start(
                out=w2_sb[:, :, 0:128], in_=w2_r[:, :, 0:128].bitcast(F32R)
            )
            nc.sync.dma_start(
                out=w2_sb[:, :, 128:D], in_=w2_r[:, :, 128:D].bitcast(F32R)
            )

            # biases ride the ACT HWDGE ring, off the critical sync FIFO
            b1_sb = consts.tile([128, FC], F32, tag="b1")
            nc.scalar.dma_start(out=b1_sb[:, :], in_=b1_r[:, :])
            b2_sb = consts.tile([128, DC], F32, tag="b2")
            nc.scalar.dma_start(out=b2_sb[:, :], in_=b2_r[:, :])

            # combine weights (host-replicated to 128 partitions); needed
            # first by tile0's dc sweep, so it follows w2 in the sync FIFO
            cwb = consts.tile([128, C], F32, tag="cw")
            nc.sync.dma_start(out=cwb[:, :], in_=cw[:, :])

            for i in range(1, len(sizes)):
                for si in range(len(tile_subs[i])):
                    _load_xt(i, si)

            for i, nt in enumerate(sizes):
                off = offs[i]
                subs = tile_subs[i]

                # h^T tile per (sub-block, F-chunk): [128 f, ssz tokens]
                h_tiles = {}
                for si, (soff, ssz) in enumerate(subs):
                    for fc in range(FC):
                        ps = ps_h.tile([128, ssz], F32, tag="psh")
                        for dc in range(DC):
                            nc.tensor.matmul(
                                ps[:, :],
                                w1_sb[:, dc, fc * 128:(fc + 1) * 128],
                                xt_sub[(i, si)][:, dc, :],
                                start=(dc == 0),
                                stop=(dc == DC - 1),
                            )
                        h = hp.tile([128, ssz], F32R, tag="h")
                        nc.scalar.activation(
                            h[:, :], ps[:, :],
                            mybir.ActivationFunctionType.Gelu_apprx_tanh,
                            bias=b1_sb[:, fc:fc + 1],
                        )
                        h_tiles[(si, fc)] = h

                for si, (soff, ssz) in enumerate(subs):
                    for dc in range(DC):
                        ps2 = ps_y.tile([128, ssz], F32, tag="psy")
                        for fc in range(FC):
                            nc.tensor.matmul(
                                ps2[:, :],
                                w2_sb[:, fc, dc * 128:(dc + 1) * 128],
                                h_tiles[(si, fc)][:, :],
                                start=(fc == 0),
                                stop=(fc == FC - 1),
                            )
                        ytmp = yp.tile([128, ssz], F32, tag="ytmp")
                        nc.scalar.activation(
                            ytmp[:, :], ps2[:, :],
                            mybir.ActivationFunctionType.Identity,
                            bias=b2_sb[:, dc:dc + 1],
                        )
                        yout = ybigp.tile([128, ssz], F32, tag="yout")
                        nc.vector.tensor_mul(
                            yout[:, :], ytmp[:, :],
                            cwb[:, off + soff:off + soff + ssz],
                        )
                        nc.sync.dma_start(
                            out=yt_r[:, dc, off + soff:off + soff + ssz],
                            in_=yout[:, :],
                        )

    nc.finalize()
    return nc


def kernel(hidden, top_k_indices, top_k_weights, W1, b1, W2, b2):
    global LAST_RESULTS
    x = np.ascontiguousarray(np.asarray(hidden, dtype=np.float32).reshape(T, D))
    idx = np.asarray(top_k_indices).reshape(T, TOPK)
    w = np.asarray(top_k_weights, dtype=np.float32).reshape(T, TOPK)
    W1 = np.asarray(W1, dtype=np.float32)
    b1 = np.asarray(b1, dtype=np.float32)
    W2 = np.asarray(W2, dtype=np.float32)
    b2 = np.asarray(b2, dtype=np.float32)

    # Host routing: token lists + combined weights per expert
    tok_lists, cw_lists = [], []
    for e in range(E):
        m = idx == e
        toks = np.nonzero(m.any(axis=1))[0]
        cw_t = (w * m).sum(axis=1)[toks]
        tok_lists.append(toks)
        cw_lists.append(cw_t)

    maxn = max(len(t) for t in tok_lists)
    C = max(512, -(-maxn // 64) * 64)

    if C not in _nc_cache:
        _nc_cache[C] = _build_nc(C)
    nc = _nc_cache[C]

    in_maps = []
    for e in range(E):
        toks = tok_lists[e]
        n = len(toks)
        xt = np.zeros((D, C), np.float32)
        xt[:, :n] = x[toks].T
        cw_arr = np.zeros((128, C), np.float32)
        cw_arr[:, :n] = cw_lists[e][None, :]
        in_maps.append({
            "xt": xt,
            "w1": np.ascontiguousarray(W1[e]),
            "b1": np.ascontiguousarray(b1[e].reshape(F, 1)),
            "w2": np.ascontiguousarray(W2[e]),
            "b2": np.ascontiguousarray(b2[e].reshape(D, 1)),
            "cw": cw_arr,
        })

    kwargs = {}
    if TRACE:
        kwargs = dict(trace=True, trace_cores=list(range(E)))
    res = run_bass_kernel_spmd(nc, in_maps, core_ids=list(range(E)), **kwargs)
    LAST_RESULTS = res

    out = np.zeros((T, D), np.float32)
    for e in range(E):
        toks = tok_lists[e]
        n = len(toks)
        out[toks] += res.results[e]["yt"][:, :n].T
    return out.reshape(B, S, D)


# revision 19
# speedup vs baseline: 1.1130x; 1.0088x over previous
"""Expert-parallel MoE layer for 8 Trainium2 NeuronCores.

Strategy: each of the 8 experts is assigned to one core. The host computes
the routing (which tokens go to which expert and with what combined weight),
gathers + transposes each expert's tokens into a padded [D, C] activation
matrix, and each core runs a fused  gelu(x @ W1 + b1) @ W2 + b2  kernel for
its expert, scaling each token's output column by the combine weight. The
host scatter-adds the per-expert outputs back into the full [B, S, D] output.

Matmuls run as float32r (full fp32 storage, reduced-precision PE multiply) at
1 column/cycle -- 4x faster than true fp32 matmul on TRN2.
"""

import sys

if "/opt/trn_rl_repo" not in sys.path:
    sys.path.insert(0, "/opt/trn_rl_repo")

import numpy as np

import concourse.bass as bass
import concourse.tile as tile
from concourse import bacc, mybir
from concourse.bass_utils import run_bass_kernel_spmd

B, S, D, F, E, TOPK = 4, 2048, 512, 1024, 8, 2
T = B * S
F32 = mybir.dt.float32
F32R = mybir.dt.float32r
BF16 = mybir.dt.bfloat16

DC = D // 128  # 4 contraction chunks for x @ W1
FC = F // 128  # 8 contraction chunks for h @ W2

# Set by test harness to capture a profile; harness-invisible otherwise.
TRACE = False
LAST_RESULTS = None

_nc_cache = {}


def _token_tiles(C):
    """Split C token columns into token-tile sizes.

    The first tile is up to 1536 columns (three 512 PSUM sub-blocks): its fc
    sweep gives the PE a long stretch of work that needs only w1 + the first
    x^T slices, which is exactly the runway the DMA engines need to land w2
    and the later x^T slices before they are consumed. Later tiles are
    ~512s; every tile stays >= 256 so f32r matmul runs at full rate.
    """
    assert C % 64 == 0 and C >= 512
    first = min(1536, C)
    rem = C - first
    if rem == 0:
        return [first]
    n = -(-rem // 512)
    sizes = [first]
    base, extra = divmod(rem, n)
    for j in range(n):
        sizes.append(base + (1 if j < extra else 0))
    assert sum(sizes) == C and all(s >= 256 or s == C for s in sizes[1:]), sizes
    return sizes


def _build_nc(C):
    nc = bacc.Bacc("TRN2", num_devices=E)

    xt = nc.dram_tensor("xt", [D, C], F32, kind="ExternalInput")
    w1 = nc.dram_tensor("w1", [D, F], F32, kind="ExternalInput")
    b1 = nc.dram_tensor("b1", [F, 1], F32, kind="ExternalInput")
    w2 = nc.dram_tensor("w2", [F, D], F32, kind="ExternalInput")
    b2 = nc.dram_tensor("b2", [D, 1], F32, kind="ExternalInput")
    cw = nc.dram_tensor("cw", [128, C], F32, kind="ExternalInput")
    yt = nc.dram_tensor("yt", [D, C], F32, kind="ExternalOutput")

    # DRAM views with the 128-partition chunk dim split out
    xt_r = xt.rearrange("(c p) t -> p c t", p=128)  # [128, DC, C]
    w1_r = w1.rearrange("(c p) f -> p c f", p=128)  # [128, DC, F]
    w2_r = w2.rearrange("(c p) d -> p c d", p=128)  # [128, FC, D]
    b1_r = b1.rearrange("(c p) o -> p (c o)", p=128)  # [128, FC]
    b2_r = b2.rearrange("(c p) o -> p (c o)", p=128)  # [128, DC]
    yt_r = yt.rearrange("(c p) t -> p c t", p=128)  # [128, DC, C]

    sizes = _token_tiles(C)

    with tile.TileContext(nc) as tc:
        with (
            tc.tile_pool(name="consts", bufs=1) as consts,
            tc.tile_pool(name="xtp", bufs=1) as xtp,
            tc.tile_pool(name="hp", bufs=26) as hp,
            tc.tile_pool(name="yp", bufs=4) as yp,
            tc.tile_pool(name="ybig", bufs=8) as ybigp,
            tc.tile_pool(name="ps_h", bufs=4, space="PSUM") as ps_h,
            tc.tile_pool(name="ps_y", bufs=4, space="PSUM") as ps_y,
        ):
            # PE warmup: the HAM clock gate keeps the PE at 1.2 GHz until
            # it has been busy ~3.4us. The PE would otherwise idle 7->15us
            # waiting for the first DMAs, then run the first ~8us of real
            # matmuls cold. These dummy matmuls (zero operands, no DMA
            # deps) warm the clock during the DMA wait for free.
            wu_w = consts.tile([128, 128], BF16, tag="wu_w")
            nc.vector.memset(wu_w[:, :], 0.0)
            wu_x = consts.tile([128, 512], BF16, tag="wu_x")
            nc.vector.memset(wu_x[:, :], 0.0)
            wu_ps = ps_h.tile([128, 512], F32, tag="psh")
            for k in range(20):
                nc.tensor.matmul(
                    wu_ps[:, :], wu_w[:, :], wu_x[:, :],
                    start=(k == 0), stop=(k == 19),
                )

            # w1: small first slice (the f-columns the first fc group needs)
            # so the first matmul gates on only 256 KB, then the rest in one
            # contiguous-run DMA at full line rate.
            w1_sb = consts.tile([128, DC, F], F32R, tag="w1")
            nc.sync.dma_start(
                out=w1_sb[:, :, 0:128], in_=w1_r[:, :, 0:128].bitcast(F32R)
            )

            # Sub-block split of every token tile (<=512 cols per PSUM bank);
            # x^T is loaded one sub-block at a time so the first matmul only
            # gates on the first 512 columns.
            offs = []
            off = 0
            tile_subs = []
            for i, nt in enumerate(sizes):
                offs.append(off)
                subs = []
                so = 0
                while so < nt:
                    subs.append((so, min(512, nt - so)))
                    so += 512
                tile_subs.append(subs)
                off += nt

            xt_sub = {}

            def _load_xt(i, si):
                soff, ssz = tile_subs[i][si]
                t = xtp.tile(
                    [128, DC, ssz], F32R, tag=f"xt{i}_{si}", name=f"xt_sb{i}_{si}"
                )
                base = offs[i] + soff
                nc.sync.dma_start(
                    out=t[:, :, :],
                    in_=xt_r[:, :, base:base + ssz].bitcast(F32R),
                )
                xt_sub[(i, si)] = t

            _load_xt(0, 0)

            # staged so each slice lands just ahead of the fc group that
            # consumes it (one monolithic DMA would stall fc1 until the
            # whole rest of w1 arrived)
            for lo, hi in ((128, 256), (256, 512), (512, 1024)):
                nc.sync.dma_start(
                    out=w1_sb[:, :, lo:hi], in_=w1_r[:, :, lo:hi].bitcast(F32R)
                )

            for si in range(1, len(tile_subs[0])):
                _load_xt(0, si)

            # w2; needed after the first fc sweep, so it goes ahead of the
            # remaining x^T tiles in the sync HWDGE FIFO. First d-chunk
            # separately (it unblocks the first dc group), then the rest.
            w2_sb = consts.tile([128, FC, D], F32R, tag="w2")
            nc.sync.dma_start(
                out=w2_sb[:, :, 0:128], in_=w2_r[:, :, 0:128].bitcast(F32R)
            )
            nc.sync.dma_start(
                out=w2_sb[:, :, 128:D], in_=w2_r[:, :, 128:D].bitcast(F32R)
            )

            # biases ride the ACT HWDGE ring, off the critical sync FIFO
            b1_sb = consts.tile([128, FC], F32, tag="b1")
            nc.scalar.dma_start(out=b1_sb[:, :], in_=b1_r[:, :])
            b2_sb = consts.tile([128, DC], F32, tag="b2")
            nc.scalar.dma_start(out=b2_sb[:, :], in_=b2_r[:, :])

            # combine weights (host-replicated to 128 partitions); needed
            # first by tile0's dc sweep, so it follows w2 in the sync FIFO
            cwb = consts.tile([128, C], F32, tag="cw")
            nc.sync.dma_start(out=cwb[:, :], in_=cw[:, :])

            for i in range(1, len(sizes)):
                for si in range(len(tile_subs[i])):
                    _load_xt(i, si)

            for i, nt in enumerate(sizes):
                off = offs[i]
                subs = tile_subs[i]

                # h^T tile per (sub-block, F-chunk): [128 f, ssz tokens]
                h_tiles = {}
                for si, (soff, ssz) in enumerate(subs):
                    for fc in range(FC):
                        ps = ps_h.tile([128, ssz], F32, tag="psh")
                        for dc in range(DC):
                            nc.tensor.matmul(
                                ps[:, :],
                                w1_sb[:, dc, fc * 128:(fc + 1) * 128],
                                xt_sub[(i, si)][:, dc, :],
                                start=(dc == 0),
                                stop=(dc == DC - 1),
                            )
                        h = hp.tile([128, ssz], F32R, tag="h")
                        nc.scalar.activation(
                            h[:, :], ps[:, :],
                            mybir.ActivationFunctionType.Gelu_apprx_tanh,
                            bias=b1_sb[:, fc:fc + 1],
                        )
                        h_tiles[(si, fc)] = h

                for si, (soff, ssz) in enumerate(subs):
                    for dc in range(DC):
                        ps2 = ps_y.tile([128, ssz], F32, tag="psy")
                        for fc in range(FC):
                            nc.tensor.matmul(
                                ps2[:, :],
                                w2_sb[:, fc, dc * 128:(dc + 1) * 128],
                                h_tiles[(si, fc)][:, :],
                                start=(fc == 0),
                                stop=(fc == FC - 1),
                            )
                        ytmp = yp.tile([128, ssz], F32, tag="ytmp")
                        nc.scalar.activation(
                            ytmp[:, :], ps2[:, :],
                            mybir.ActivationFunctionType.Identity,
                            bias=b2_sb[:, dc:dc + 1],
                        )
                        yout = ybigp.tile([128, ssz], F32, tag="yout")
                        nc.vector.tensor_mul(
                            yout[:, :], ytmp[:, :],
                            cwb[:, off + soff:off + soff + ssz],
                        )
                        nc.sync.dma_start(
                            out=yt_r[:, dc, off + soff:off + soff + ssz],
                            in_=yout[:, :],
                        )

    nc.finalize()
    return nc


def kernel(hidden, top_k_indices, top_k_weights, W1, b1, W2, b2):
    global LAST_RESULTS
    x = np.ascontiguousarray(np.asarray(hidden, dtype=np.float32).reshape(T, D))
    idx = np.asarray(top_k_indices).reshape(T, TOPK)
    w = np.asarray(top_k_weights, dtype=np.float32).reshape(T, TOPK)
    W1 = np.asarray(W1, dtype=np.float32)
    b1 = np.asarray(b1, dtype=np.float32)
    W2 = np.asarray(W2, dtype=np.float32)
    b2 = np.asarray(b2, dtype=np.float32)

    # Host routing: token lists + combined weights per expert
    tok_lists, cw_lists = [], []
    for e in range(E):
        m = idx == e
        toks = np.nonzero(m.any(axis=1))[0]
        cw_t = (w * m).sum(axis=1)[toks]
        tok_lists.append(toks)
        cw_lists.append(cw_t)

    maxn = max(len(t) for t in tok_lists)
    C = max(512, -(-maxn // 64) * 64)

    if C not in _nc_cache:
        _nc_cache[C] = _build_nc(C)
    nc = _nc_cache[C]

    in_maps = []
    for e in range(E):
        toks = tok_lists[e]
        n = len(toks)
        xt = np.zeros((D, C), np.float32)
        xt[:, :n] = x[toks].T
        cw_arr = np.zeros((128, C), np.float32)
        cw_arr[:, :n] = cw_lists[e][None, :]
        in_maps.append({
            "xt": xt,
            "w1": np.ascontiguousarray(W1[e]),
            "b1": np.ascontiguousarray(b1[e].reshape(F, 1)),
            "w2": np.ascontiguousarray(W2[e]),
            "b2": np.ascontiguousarray(b2[e].reshape(D, 1)),
            "cw": cw_arr,
        })

    kwargs = {}
    if TRACE:
        kwargs = dict(trace=True, trace_cores=list(range(E)))
    res = run_bass_kernel_spmd(nc, in_maps, core_ids=list(range(E)), **kwargs)
    LAST_RESULTS = res

    out = np.zeros((T, D), np.float32)
    for e in range(E):
        toks = tok_lists[e]
        n = len(toks)
        out[toks] += res.results[e]["yt"][:, :n].T
    return out.reshape(B, S, D)
